# revision 1
# baseline (speedup 1.0000x reference)
"""Trainium2 Bass kernel for nn_CapsuleLayer (dynamic routing).

Problem:  u_hat = einsum('bri,crio->cbro', x, W);  3 routing iterations
          (softmax over R, weighted sum, squash, agreement update).
Shapes:   x [256, 1152, 8] f32, W [10, 1152, 8, 16] f32 ->
          out [10, 256, 1, 1, 16] f32.

Strategy (8 NeuronCores, data-parallel over batch, B_loc = 32/core):
  * never materialize u_hat (189 MB) in HBM;
  * s-sums   : PE matmuls, K = 128-row r-blocks, i via 8 accumulating
               matmuls, y = softmax-weights * x built on DVE/GPSIMD;
  * agreement: PE matmuls with block-diagonal v stationaries streaming a
               (c,o)-partition W copy (streamed from HBM per iteration),
               then fused multiply + i-tree, L accumulated in fp32;
  * softmax  : ACT exp with accumulated Z, weights transposed to r-block
               partitions with PE transposes.
All arithmetic fp32 (bf16 anywhere in the logit path measurably breaks
the output tolerance: ~1e-2 rel err per bf16-rounded component).
"""

import sys
from contextlib import ExitStack

import numpy as np

sys.path.insert(0, "/opt/trn_rl_repo")

import concourse.bacc as bacc
import concourse.bass as bass
import concourse.mybir as mybir
import concourse.tile as tile
from concourse.bass_utils import run_bass_kernel_spmd

F32 = mybir.dt.float32
F16 = mybir.dt.float16
MUL = mybir.AluOpType.mult
ADD = mybir.AluOpType.add

B, R, I, C, O = 256, 1152, 8, 10, 16
NC = 8
BL = B // NC          # 32 batch per core
Q = R // 128          # 9 r-blocks of 128
CO = C * O            # 160
RI = R * I            # 9216
EPS = 1e-7
GCH = 1024            # g-matmul free-dim chunk (elements of (r,i))
NG = RI // GCH        # 9 chunks
W3 = 3                # (c,b) waves


def build_nc(debug=False):
    nc = bacc.Bacc("TRN2", target_bir_lowering=False, debug=debug)

    xtr_d = nc.declare_dram_parameter("xtr", [128, Q, I, BL], F32, isOutput=False)
    wfr_d = nc.declare_dram_parameter("wfr", [128, Q, I, CO], F32, isOutput=False)
    wt_d = nc.declare_dram_parameter("wt", [4, 16, 3, RI], F16, isOutput=False)
    xrep_d = nc.declare_dram_parameter("xrep", [128, RI], F16, isOutput=False)
    ident_d = nc.declare_dram_parameter("ident", [128, 128], F32, isOutput=False)
    out_d = nc.declare_dram_parameter("out", [C, O, BL], F32, isOutput=True)

    with tile.TileContext(nc) as tc, ExitStack() as ctx:
        res = ctx.enter_context(tc.tile_pool(name="res", bufs=1))
        cwp = ctx.enter_context(tc.tile_pool(name="cwp", bufs=2))
        yp = ctx.enter_context(tc.tile_pool(name="yp", bufs=2))
        wtp = ctx.enter_context(tc.tile_pool(name="wtp", bufs=2))
        gmp = ctx.enter_context(tc.tile_pool(name="gmp", bufs=2))
        trp = ctx.enter_context(tc.tile_pool(name="trp", bufs=2))
        smp = ctx.enter_context(tc.tile_pool(name="smp", bufs=1))
        psS = ctx.enter_context(
            tc.tile_pool(name="psS", bufs=1, space=bass.MemorySpace.PSUM)
        )
        psG = ctx.enter_context(
            tc.tile_pool(name="psG", bufs=2, space=bass.MemorySpace.PSUM)
        )
        psT = ctx.enter_context(
            tc.tile_pool(name="psT", bufs=2, space=bass.MemorySpace.PSUM)
        )
        psN = ctx.enter_context(
            tc.tile_pool(name="psN", bufs=1, space=bass.MemorySpace.PSUM)
        )

        # ---- resident tensors -------------------------------------
        xtr = res.tile([128, Q, I, BL], F32)
        wfr = res.tile([128, Q, I, CO], F32)
        xrep = res.tile([128, RI], F16)
        ident = res.tile([128, 128], F32)
        L = res.tile([128, W3, R], F32)
        cwT = res.tile([128, Q, W3, 128], F32)
        Z = res.tile([128, W3], F32)
        Zi = res.tile([128, W3], F32)
        vblk = res.tile([128, 3, 32], F16)   # v[c, b, o] staged at rows 32P+o
        ones16 = res.tile([16, 1], F32)
        v_sb = res.tile([16, C, BL], F32)    # squash output, [o, c, b]

        nc.sync.dma_start(xtr[:], xtr_d[:])
        nc.sync.dma_start(wfr[:], wfr_d[:])
        nc.sync.dma_start(xrep[:], xrep_d[:])
        nc.sync.dma_start(ident[:], ident_d[:])
        nc.vector.memset(L[:], 0.0)
        nc.vector.memset(ones16[:], 1.0)

        # ---------------------------------------------------------------
        def build_y(c):
            """y_c[rr, q, i, b] = cw[c, b, 128q+rr] * x[b, 128q+rr, i].

            cw comes from cwT (r-block partitions); (c,b) column index in
            cwT is p = 32k + b, where class c = 4w + k (w=2: c = 8 + k).
            """
            w = min(c // 4, 2)
            k = c - 4 * w
            y_c = yp.tile([128, Q, I, BL], F32, tag="y")
            cw_src = (
                cwT[:, :, w, 32 * k : 32 * k + 32]
                .unsqueeze(2)
                .broadcast_to([128, Q, I, BL])
            )
            eng = nc.vector if c % 2 == 0 else nc.gpsimd
            eng.tensor_tensor(y_c, xtr[:], cw_src, MUL)
            return y_c

        def s_pass(it):
            """sps[o, c, b] = sum_{r,i} rhs_c[r, i, b] * W[c, r, i, o]."""
            sps = psS.tile([16, C, BL], F32, tag="sps")
            for c in range(C):
                y_c = None if it == 0 else build_y(c)
                for q in range(Q):
                    for i in range(I):
                        rhs = xtr[:, q, i, :] if it == 0 else y_c[:, q, i, :]
                        nc.tensor.matmul(
                            sps[:, c, :],
                            wfr[:, q, i, c * O : (c + 1) * O],
                            rhs,
                            start=(q == 0 and i == 0),
                            stop=(q == Q - 1 and i == I - 1),
                        )
            return sps

        # ---------------------------------------------------------------
        def squash(sps, it):
            """v_sb = squash(s) over o;  it==0 folds the uniform 1/R weight."""
            sq = smp.tile([16, C * BL], F32, tag="sq")
            nc.scalar.activation(
                sq[:],
                sps[:].rearrange("o c b -> o (c b)"),
                mybir.ActivationFunctionType.Square,
            )
            snps = psN.tile([1, C * BL], F32, tag="snps")
            nc.tensor.matmul(snps[:], ones16[:], sq[:], start=True, stop=True)
            sn = smp.tile([1, C * BL], F32, tag="sn")
            if it == 0:
                nc.vector.tensor_scalar_mul(sn[:], snps[:], 1.0 / (R * R))
            else:
                nc.vector.tensor_copy(sn[:], snps[:])
            u1 = smp.tile([1, C * BL], F32, tag="u1")
            u2 = smp.tile([1, C * BL], F32, tag="u2")
            u3 = smp.tile([1, C * BL], F32, tag="u3")
            u4 = smp.tile([1, C * BL], F32, tag="u4")
            f = smp.tile([1, C * BL], F32, tag="f")
            nc.vector.tensor_scalar_add(u1[:], sn[:], EPS)
            nc.scalar.activation(u2[:], u1[:], mybir.ActivationFunctionType.Sqrt)
            nc.vector.tensor_scalar_add(u3[:], sn[:], 1.0)
            nc.vector.tensor_tensor(u4[:], u2[:], u3[:], MUL)
            nc.vector.reciprocal(u1[:], u4[:])
            nc.vector.tensor_tensor(f[:], sn[:], u1[:], MUL)
            if it == 0:
                nc.vector.tensor_scalar_mul(f[:], f[:], 1.0 / R)
            frep = smp.tile([16, C * BL], F32, tag="frep")
            nc.gpsimd.partition_broadcast(frep[:], f[:])
            nc.vector.tensor_tensor(
                v_sb[:].rearrange("o c b -> o (c b)"),
                sps[:].rearrange("o c b -> o (c b)"),
                frep[:],
                MUL,
            )

        def fill_vblk():
            """Stage v as g-matmul stationaries (fp16): slot cc of strip P
            holds class 2P+cc; slot 2 of strip P holds class 8+P (P<2)."""
            for c in range(C):
                P, slot = (c // 2, c % 2) if c < 8 else (c - 8, 2)
                nc.vector.tensor_copy(
                    vblk[32 * P : 32 * P + 16, slot, :], v_sb[:, c, :]
                )

        # ---------------------------------------------------------------
        def agreement():
            """L[p, w, r] += sum_o v[c,b,o]*W[c,r,i,o] (*) x[b,r,i], sum_i."""
            for n0 in range(NG):
                off = n0 * GCH
                wt_t = wtp.tile([128, 3, GCH], F16, tag="wt")
                for P in range(4):
                    nc.sync.dma_start(
                        wt_t[32 * P : 32 * P + 16, :, :],
                        wt_d[P, :, :, off : off + GCH],
                    )
                for w in range(W3):
                    gps = psG.tile([128, GCH], F32, tag="gps")
                    npart = 128 if w < 2 else 64
                    for k in range(4 if w < 2 else 2):
                        c = 4 * w + k
                        P, slot = (c // 2, c % 2) if c < 8 else (c - 8, 2)
                        base = 32 * P
                        for sub in range(0, GCH, 512):
                            nc.tensor.matmul(
                                gps[32 * k : 32 * k + 32, sub : sub + 512],
                                vblk[base : base + 16, slot, :],
                                wt_t[base : base + 16, slot, sub : sub + 512],
                                start=True,
                                stop=True,
                                tile_position=(base, 32 * k),
                            )
                    # fused drain-multiply:  gm = gps * xrep   (fp32, DVE)
                    gm = gmp.tile([128, GCH], F32, tag="gm")
                    nc.vector.tensor_tensor(
                        gm[:npart, :],
                        gps[:npart, :],
                        xrep[:npart, off : off + GCH],
                        MUL,
                    )
                    # i-reduction tree (8 -> 4 -> 2 -> 1) on GPSIMD
                    nr = GCH // I
                    l1 = trp.tile([128, GCH // 2], F32, tag="l1")
                    l2 = trp.tile([128, GCH // 4], F32, tag="l2")
                    a = trp.tile([128, GCH // 8], F32, tag="a")
                    gmv = gm.rearrange("p (r i) -> p r i", i=I)
                    l1v = l1.rearrange("p (r i) -> p r i", i=4)
                    l2v = l2.rearrange("p (r i) -> p r i", i=2)
                    nc.gpsimd.tensor_tensor(
                        l1v[:npart], gmv[:npart, :, 0:4], gmv[:npart, :, 4:8], ADD
                    )
                    nc.gpsimd.tensor_tensor(
                        l2v[:npart], l1v[:npart, :, 0:2], l1v[:npart, :, 2:4], ADD
                    )
                    nc.gpsimd.tensor_tensor(
                        a[:npart], l2v[:npart, :, 0], l2v[:npart, :, 1], ADD
                    )
                    r0 = off // I
                    nc.vector.tensor_tensor(
                        L[:npart, w, r0 : r0 + nr],
                        L[:npart, w, r0 : r0 + nr],
                        a[:npart],
                        ADD,
                    )

        # ---------------------------------------------------------------
        def softmax_transpose():
            """cw = softmax_r(L) per (c,b); write transposed into cwT."""
            for w in range(W3):
                cwv = cwp.tile([128, R], F32, tag="cw")
                nc.scalar.activation(
                    cwv[:],
                    L[:, w, :],
                    mybir.ActivationFunctionType.Exp,
                    accum_out=Z[:, w : w + 1],
                )
                nc.vector.reciprocal(Zi[:, w : w + 1], Z[:, w : w + 1])
                nc.vector.tensor_scalar_mul(cwv[:], cwv[:], Zi[:, w : w + 1])
                for q in range(Q):
                    tps = psT.tile([128, 128], F32, tag="tps")
                    nc.tensor.transpose(
                        tps[:], cwv[:, 128 * q : 128 * (q + 1)], ident[:]
                    )
                    nc.scalar.copy(cwT[:, q, w, :], tps[:])

        # =========================== flow ==============================
        for it in range(3):
            if it > 0:
                softmax_transpose()
            sps = s_pass(it)
            squash(sps, it)
            if it < 2:
                fill_vblk()
                agreement()

        nc.sync.dma_start(out_d[:].rearrange("c o b -> o c b"), v_sb[:])

    nc.compile()
    return nc


# =================== host-side prep / entry point =====================

def _prep_shared(W):
    """Per-problem constant tensors (replicated on every core)."""
    W = np.ascontiguousarray(W, np.float32)
    # wfr[rr, q, i, 16c+o] = W[c, 128q+rr, i, o]
    wfr = np.ascontiguousarray(
        W.reshape(C, Q, 128, I, O).transpose(2, 1, 3, 0, 4).reshape(128, Q, I, CO)
    )
    # wt[P, o, slot, 8r+i]: slot cc<2 -> W[2P+cc]; slot 2 -> W[8+P] (P<2).
    wt = np.zeros((4, 16, 3, RI), np.float16)
    for P in range(4):
        for cc in range(2):
            wt[P, :, cc, :] = W[2 * P + cc].transpose(2, 0, 1).reshape(O, RI)
    for P in range(2):
        wt[P, :, 2, :] = W[8 + P].transpose(2, 0, 1).reshape(O, RI)
    ident = np.eye(128, dtype=np.float32)
    return wfr, wt, ident


def _prep_core(x_shard):
    """Per-core tensors for one 32-batch shard: xtr and xrep."""
    xs = np.ascontiguousarray(x_shard, np.float32)       # [32, 1152, 8]
    xtr = np.ascontiguousarray(
        xs.reshape(BL, Q, 128, I).transpose(2, 1, 3, 0)
    )                                                     # [128, Q, I, 32]
    flat = xs.reshape(BL, RI)                             # [b, 8r+i]
    xrep = np.ascontiguousarray(
        flat[np.arange(128) % BL].astype(np.float16)
    )                                                     # [128, RI]
    return xtr, xrep


_NC_CACHE = {}


def kernel(x, W):
    x = np.asarray(x, np.float32)
    W = np.asarray(W, np.float32)
    if "nc" not in _NC_CACHE:
        _NC_CACHE["nc"] = build_nc()
    nc = _NC_CACHE["nc"]

    wfr, wt, ident = _prep_shared(W)
    in_maps = []
    for m in range(NC):
        xtr, xrep = _prep_core(x[m * BL : (m + 1) * BL])
        in_maps.append(
            {"xtr": xtr, "wfr": wfr, "wt": wt, "xrep": xrep, "ident": ident}
        )

    res = run_bass_kernel_spmd(nc, in_maps, list(range(NC)))
    out = np.empty((C, B, 1, 1, O), np.float32)
    for m in range(NC):
        o = res.results[m]["out"]                         # [C, O, BL]
        out[:, m * BL : (m + 1) * BL, 0, 0, :] = np.asarray(o).transpose(0, 2, 1)
    return out


if __name__ == "__main__":
    d = np.load("/root/problem/ref_data.npz")
    got = kernel(d["x"], d["W"])
    exp = d["expected"]
    err = np.abs(got - exp).max() / np.abs(exp).max()
    print("Relative error:", err)



# revision 12
# speedup vs baseline: 1.3417x; 1.3417x over previous
"""Trainium2 Bass kernel for nn_CapsuleLayer (dynamic routing).

Problem:  u_hat = einsum('bri,crio->cbro', x, W);  3 routing iterations
          (softmax over R, weighted sum, squash, agreement update).
Shapes:   x [256, 1152, 8] f32, W [10, 1152, 8, 16] f32 ->
          out [10, 256, 1, 1, 16] f32.

Strategy (8 NeuronCores, data-parallel over batch, B_loc = 32/core):
  * never materialize u_hat in HBM; all W layouts fp16, loaded once.
  * s-pass: stationary = wfrp 4-class group [128,128] (each class in a
    32-col slot so PSUM diagonal blocks are 32-partition aligned),
    moving = y = cw*x fp16 [128,128]; it0 reuses the same matmuls with
    a broadcast x moving operand (uniform routing weights folded into
    the squash scalars).  216 matmuls / pass, 320 fp16 cols per (q,i).
  * agreement: one block-diagonal [128,128] fp16 stationary per
    (c,b)-wave (v embedded on device), moving = wt2 [128,1024] chunks
    (resident in SBUF); i-reduction via a single DVE tensor_reduce
    straight into L.
  * softmax: ACT exp with accumulated Z, PE transposes -> cwT fp16.
Logit-path partial sums stay fp32 (products only rounded to fp16).
"""

import sys
from contextlib import ExitStack

import numpy as np

sys.path.insert(0, "/opt/trn_rl_repo")

import concourse.bacc as bacc
import concourse.bass as bass
import concourse.mybir as mybir
import concourse.tile as tile
from concourse.bass_utils import run_bass_kernel_spmd

F32 = mybir.dt.float32
F16 = mybir.dt.float16
MUL = mybir.AluOpType.mult
ADD = mybir.AluOpType.add
AXX = mybir.AxisListType.X

B, R, I, C, O = 256, 1152, 8, 10, 16
NC = 8
BL = B // NC          # 32 batch per core
Q = R // 128          # 9 r-blocks of 128
CO = C * O            # 160
CB = C * BL           # 320
RI = R * I            # 9216
PW = 320              # padded stationary width: g0,g1 128 (4 cls), g2 64
EPS = 1e-7
GCH = 1024            # agreement chunk (elements of (r,i)) = 128 r
NG = RI // GCH        # 9 chunks
W3 = 3                # (c,b) waves: w0 = c0-3, w1 = c4-7, w2 = c8-9
G3 = 3                # s-pass class groups: g0 = c0-3, g1 = c4-7, g2 = c8-9


def build_nc(debug=False):
    nc = bacc.Bacc("TRN2", target_bir_lowering=False, debug=debug)

    xtr_d = nc.declare_dram_parameter("xtr", [128, Q, I, BL], F16, isOutput=False)
    wfrp_d = nc.declare_dram_parameter("wfrp", [128, Q, I, PW], F16, isOutput=False)
    w2a_d = nc.declare_dram_parameter("w2a", [128, 2, RI], F16, isOutput=False)
    w2b_d = nc.declare_dram_parameter("w2b", [64, RI], F16, isOutput=False)
    xrep_d = nc.declare_dram_parameter("xrep", [128, RI], F16, isOutput=False)
    ident_d = nc.declare_dram_parameter("ident", [128, 128], F32, isOutput=False)
    out_d = nc.declare_dram_parameter("out", [C, O, BL], F32, isOutput=True)
    if debug:
        dbg_v0_d = nc.declare_dram_parameter("dbg_v0", [16, C, BL], F32, isOutput=True)
        dbg_L_d = nc.declare_dram_parameter("dbg_L", [128, W3, R], F32, isOutput=True)
        dbg_cwT_d = nc.declare_dram_parameter("dbg_cwT", [128, Q, CB], F16, isOutput=True)
        dbg_s1_d = nc.declare_dram_parameter("dbg_s1", [16, C, BL], F32, isOutput=True)

    with tile.TileContext(nc) as tc, ExitStack() as ctx:
        res = ctx.enter_context(tc.tile_pool(name="res", bufs=1))
        yp = ctx.enter_context(tc.tile_pool(name="yp", bufs=2))
        gmp = ctx.enter_context(tc.tile_pool(name="gmp", bufs=2))
        cwp = ctx.enter_context(tc.tile_pool(name="cwp", bufs=1))
        smp = ctx.enter_context(tc.tile_pool(name="smp", bufs=1))
        psS = ctx.enter_context(
            tc.tile_pool(name="psS", bufs=1, space=bass.MemorySpace.PSUM)
        )
        psG = ctx.enter_context(
            tc.tile_pool(name="psG", bufs=2, space=bass.MemorySpace.PSUM)
        )
        psT = ctx.enter_context(
            tc.tile_pool(name="psT", bufs=2, space=bass.MemorySpace.PSUM)
        )
        psN = ctx.enter_context(
            tc.tile_pool(name="psN", bufs=1, space=bass.MemorySpace.PSUM)
        )

        # ---- resident tensors -------------------------------------
        xtr = res.tile([128, Q, I, BL], F16)
        wfrp = res.tile([128, Q, I, PW], F16)
        w2a = res.tile([128, 2, RI], F16)
        w2b = res.tile([64, RI], F16)
        xrep = res.tile([128, RI], F16)
        ident = res.tile([128, 128], F32)
        L = res.tile([128, W3, R], F32)
        S = res.tile([128, W3, 128], F16)    # agreement stationaries
        cwT = res.tile([128, Q, CB], F16)    # softmax weights, [rr, q, 32c+b]
        sps = res.tile([16, C, BL], F32)     # gathered s, [o, c, b]
        v_sb = res.tile([16, C, BL], F32)    # squash output, [o, c, b]
        Z = res.tile([128, W3], F32)
        Zi = res.tile([128, W3], F32)
        ones16 = res.tile([16, 1], F32)

        # ---- input DMAs (ordered to overlap with it0 compute) -----
        nc.sync.dma_start(ident[:], ident_d[:])
        nc.sync.dma_start(xtr[:], xtr_d[:])
        for q in range(Q):
            nc.sync.dma_start(wfrp[:, q], wfrp_d[:, q])
        nc.sync.dma_start(xrep[:], xrep_d[:])
        SEG = RI // 3
        for w in range(2):
            for s3 in range(3):
                nc.sync.dma_start(
                    w2a[:, w, s3 * SEG : (s3 + 1) * SEG],
                    w2a_d[:, w, s3 * SEG : (s3 + 1) * SEG],
                )
        for s3 in range(3):
            nc.sync.dma_start(
                w2b[:, s3 * SEG : (s3 + 1) * SEG],
                w2b_d[:, s3 * SEG : (s3 + 1) * SEG],
            )
        nc.vector.memset(S[:], 0.0)
        nc.vector.memset(ones16[:], 1.0)

        # ---------------------------------------------------------------
        def fill_S():
            """S[32k+o, w, 32k+b] = v[4w+k, b, o] (fp32->fp16 copies)."""
            for c in range(C):
                w, k = (c // 4, c % 4) if c < 8 else (2, c - 8)
                nc.scalar.copy(
                    S[32 * k : 32 * k + 16, w, 32 * k : 32 * k + 32],
                    v_sb[:, c, :],
                )

        # ---------------------------------------------------------------
        def s_pass(it):
            """sps[o,c,b] = sum_{r,i} W * cw * x; 4-class groups, 32-col
            class slots so diagonal PSUM blocks are 32-aligned."""
            spsB = psS.tile([128, G3, 128], F32, tag="spsB")
            # start=True zeroes a whole PSUM bank, which would wipe the
            # other groups' first partials -- memset once, accumulate all.
            nc.vector.memset(spsB[:], 0.0)
            for q in range(Q):
                y_q = yp.tile([128, I, C, BL], F16, tag="y")
                xb = xtr[:, q, :, :].unsqueeze(2).broadcast_to([128, I, C, BL])
                if it > 0:
                    nc.vector.tensor_tensor(
                        y_q[:],
                        xb,
                        cwT[:, q, :]
                        .rearrange("p (c b) -> p c b", b=BL)
                        .unsqueeze(1)
                        .broadcast_to([128, I, C, BL]),
                        MUL,
                    )
                else:
                    nc.vector.tensor_copy(y_q[:], xb)
                for i in range(I):
                    for g in range(G3):
                        ncls = 4 if g < 2 else 2
                        nc.tensor.matmul(
                            spsB[: 32 * ncls, g, : 32 * ncls],
                            wfrp[:, q, i, 128 * g : 128 * g + 32 * ncls],
                            y_q[:, i, 4 * g : 4 * g + ncls, :],
                            start=False,
                            stop=(q == Q - 1 and i == I - 1),
                            skip_group_check=True,
                        )
            for g in range(G3):
                for cc in range(4 if g < 2 else 2):
                    nc.scalar.copy(
                        sps[:, 4 * g + cc, :],
                        spsB[32 * cc : 32 * cc + 16, g, 32 * cc : 32 * cc + 32],
                    )

        # ---------------------------------------------------------------
        def squash(it):
            """v_sb = squash(s) over o; it==0 folds the uniform 1/R."""
            sq = smp.tile([16, CB], F32, tag="sq")
            nc.scalar.activation(
                sq[:],
                sps[:].rearrange("o c b -> o (c b)"),
                mybir.ActivationFunctionType.Square,
            )
            snps = psN.tile([1, CB], F32, tag="snps")
            nc.tensor.matmul(snps[:], ones16[:], sq[:], start=True, stop=True)
            sn = smp.tile([1, CB], F32, tag="sn")
            if it == 0:
                nc.vector.tensor_scalar_mul(sn[:], snps[:], 1.0 / (R * R))
            else:
                nc.vector.tensor_copy(sn[:], snps[:])
            u1 = smp.tile([1, CB], F32, tag="u1")
            u2 = smp.tile([1, CB], F32, tag="u2")
            u3 = smp.tile([1, CB], F32, tag="u3")
            f = smp.tile([1, CB], F32, tag="f")
            nc.vector.tensor_scalar_add(u1[:], sn[:], EPS)
            nc.scalar.activation(u2[:], u1[:], mybir.ActivationFunctionType.Sqrt)
            nc.vector.tensor_scalar_add(u3[:], sn[:], 1.0)
            nc.vector.tensor_tensor(u1[:], u2[:], u3[:], MUL)
            nc.vector.reciprocal(u2[:], u1[:])
            nc.vector.tensor_tensor(f[:], sn[:], u2[:], MUL)
            if it == 0:
                nc.vector.tensor_scalar_mul(f[:], f[:], 1.0 / R)
            frep = smp.tile([16, CB], F32, tag="frep")
            nc.gpsimd.partition_broadcast(frep[:], f[:])
            nc.vector.tensor_tensor(
                v_sb[:].rearrange("o c b -> o (c b)"),
                sps[:].rearrange("o c b -> o (c b)"),
                frep[:],
                MUL,
            )

        # ---------------------------------------------------------------
        def agreement(acc):
            """L[p=(k,b), w, r] (+)= sum_{i,o} v*W*x via PE + DVE reduce.
            acc=True adds to the previous iteration's logits (reference
            accumulates b across routing iterations)."""
            for w in range(W3):
                npart = 128 if w < 2 else 64
                for n0 in range(NG):
                    off = n0 * GCH
                    gps = psG.tile([128, GCH], F32, tag="gps")
                    for sub in range(0, GCH, 512):
                        if w < 2:
                            nc.tensor.matmul(
                                gps[:, sub : sub + 512],
                                S[:, w, :],
                                w2a[:, w, off + sub : off + sub + 512],
                                start=True,
                                stop=True,
                            )
                        else:
                            nc.tensor.matmul(
                                gps[:64, sub : sub + 512],
                                S[:64, 2, :64],
                                w2b[:, off + sub : off + sub + 512],
                                start=True,
                                stop=True,
                            )
                    gm = gmp.tile([128, GCH], F16, tag="gm")
                    nc.vector.tensor_tensor(
                        gm[:npart], gps[:npart], xrep[:npart, off : off + GCH], MUL
                    )
                    if not acc:
                        nc.vector.tensor_reduce(
                            L[:npart, w, 128 * n0 : 128 * (n0 + 1)],
                            gm[:npart].rearrange("p (r i) -> p r i", i=I),
                            AXX,
                            ADD,
                        )
                    else:
                        red = gmp.tile([128, 128], F32, tag="red")
                        nc.vector.tensor_reduce(
                            red[:npart],
                            gm[:npart].rearrange("p (r i) -> p r i", i=I),
                            AXX,
                            ADD,
                        )
                        nc.vector.tensor_tensor(
                            L[:npart, w, 128 * n0 : 128 * (n0 + 1)],
                            L[:npart, w, 128 * n0 : 128 * (n0 + 1)],
                            red[:npart],
                            ADD,
                        )

        # ---------------------------------------------------------------
        def softmax_transpose():
            """cw = softmax_r(L) per (c,b); write transposed fp16 cwT."""
            cws = []
            for w in range(W3):
                npart = 128 if w < 2 else 64
                cwv = cwp.tile([128, R], F32, tag=f"cw{w}")
                cws.append(cwv)
                nc.scalar.activation(
                    cwv[:npart],
                    L[:npart, w, :],
                    mybir.ActivationFunctionType.Exp,
                    accum_out=Z[:npart, w : w + 1],
                )
                nc.vector.reciprocal(Zi[:npart, w : w + 1], Z[:npart, w : w + 1])
                nc.vector.tensor_scalar_mul(
                    cwv[:npart], cwv[:npart], Zi[:npart, w : w + 1]
                )
            for q in range(Q):
                for w in range(W3):
                    npart = 128 if w < 2 else 64
                    tps = psT.tile([128, 128], F32, tag="tps")
                    nc.tensor.transpose(
                        tps[:, :npart],
                        cws[w][:npart, 128 * q : 128 * (q + 1)],
                        ident[:npart, :npart],
                    )
                    nc.scalar.copy(
                        cwT[:, q, 128 * w : 128 * w + npart], tps[:, :npart]
                    )

        # =========================== flow ==============================
        for it in range(3):
            if it > 0:
                softmax_transpose()
                if debug and it == 1:
                    nc.sync.dma_start(dbg_cwT_d[:], cwT[:])
            s_pass(it)
            if debug and it == 1:
                nc.sync.dma_start(dbg_s1_d[:], sps[:])
            squash(it)
            if debug and it == 0:
                nc.sync.dma_start(dbg_v0_d[:], v_sb[:])
            if it < 2:
                fill_S()
                agreement(acc=(it > 0))
                if debug and it == 0:
                    nc.sync.dma_start(dbg_L_d[:], L[:])

        nc.sync.dma_start(out_d[:].rearrange("c o b -> o c b"), v_sb[:])

    nc.compile()
    return nc


# =================== host-side prep / entry point =====================

def _prep_shared(W):
    """Per-problem constant tensors (replicated on every core)."""
    W = np.ascontiguousarray(W, np.float32)
    # wfrp[rr, q, i, 128g + 32cc + o] = W[4g+cc, 128q+rr, i, o]
    wfrp = np.zeros((128, Q, 128, I, O), np.float16)  # temp [rr,q,32-slot? ]
    wfrp = np.zeros((128, Q, I, PW), np.float16)
    wr = W.reshape(C, Q, 128, I, O).transpose(2, 1, 3, 0, 4)  # [rr,q,i,c,o]
    for c in range(C):
        g, cc = (c // 4, c % 4) if c < 8 else (2, c - 8)
        wfrp[:, :, :, 128 * g + 32 * cc : 128 * g + 32 * cc + O] = wr[:, :, :, c]
    # w2a[32k+o, w, 8r+i] = W[4w+k, r, i, o];  w2b[32k+o, 8r+i] = W[8+k,...]
    w2a = np.zeros((128, 2, RI), np.float16)
    for w in range(2):
        for k in range(4):
            w2a[32 * k : 32 * k + 16, w, :] = (
                W[4 * w + k].transpose(2, 0, 1).reshape(O, RI)
            )
    w2b = np.zeros((64, RI), np.float16)
    for k in range(2):
        w2b[32 * k : 32 * k + 16, :] = W[8 + k].transpose(2, 0, 1).reshape(O, RI)
    ident = np.eye(128, dtype=np.float32)
    return wfrp, w2a, w2b, ident


def _prep_core(x_shard):
    """Per-core tensors for one 32-batch shard: xtr and xrep."""
    xs = np.ascontiguousarray(x_shard, np.float32)       # [32, 1152, 8]
    xtr = np.ascontiguousarray(
        xs.reshape(BL, Q, 128, I).transpose(2, 1, 3, 0)
    ).astype(np.float16)                                  # [128, Q, I, 32]
    flat = xs.reshape(BL, RI)                             # [b, 8r+i]
    xrep = np.ascontiguousarray(
        flat[np.arange(128) % BL].astype(np.float16)
    )                                                     # [128, RI]
    return xtr, xrep


def prep_in_maps(x, W):
    wfrp, w2a, w2b, ident = _prep_shared(W)
    in_maps = []
    for m in range(NC):
        xtr, xrep = _prep_core(x[m * BL : (m + 1) * BL])
        in_maps.append(
            {
                "xtr": xtr,
                "wfrp": wfrp,
                "w2a": w2a,
                "w2b": w2b,
                "xrep": xrep,
                "ident": ident,
            }
        )
    return in_maps


_NC_CACHE = {}


def kernel(x, W):
    x = np.asarray(x, np.float32)
    W = np.asarray(W, np.float32)
    if "nc" not in _NC_CACHE:
        _NC_CACHE["nc"] = build_nc()
    nc = _NC_CACHE["nc"]

    res = run_bass_kernel_spmd(nc, prep_in_maps(x, W), list(range(NC)))
    out = np.empty((C, B, 1, 1, O), np.float32)
    for m in range(NC):
        o = res.results[m]["out"]                         # [C, O, BL]
        out[:, m * BL : (m + 1) * BL, 0, 0, :] = np.asarray(o).transpose(0, 2, 1)
    return out


if __name__ == "__main__":
    d = np.load("/root/problem/ref_data.npz")
    got = kernel(d["x"], d["W"])
    exp = d["expected"]
    err = np.abs(got - exp).max() / np.abs(exp).max()
    print("Relative error:", err)
